# revision 17
# baseline (speedup 1.0000x reference)
"""Bar-level attention Trainium2 kernel (8 NeuronCores, head-parallel).

Contract: kernel(**inputs) takes the FULL inputs from setup_inputs() and
returns the FULL [1, 2048, 512] float32 output.

Strategy (one head per core, 8 heads / 8 cores), v2 layout:
  - Host: XT [512, 2048] bf16; one packed weight wall per head
    ([WqT*scale | WkT] | WvT | WoT) bf16; sigmoid(gate) folded into
    per-partition scalars; bq/bk are zero in setup_inputs (guarded);
    bv/bo folded into the host-side bias add (exact: softmax rows sum
    to 1, so the V bias passes straight through both branches).
  - Device (per core):
      Q^T/K^T [64, 2048] f32r (joint [128, *] projection, split on copy),
      V [k, 65] bf16 per 128-key chunk (col 64 = ones -> denominators).
      Attention per 1024-query half, per 128-key chunk:
        S^T = K_chunk @ Q^T -> psum [128, 1024] (keys on partitions)
        E = exp(S^T) -> bf16 (no max subtraction: scores ~ N(0,1))
        global AV: acc[q_sub 128, 65] += E_sub^T @ V (8 query subtiles,
          full 128-partition contraction; G/L accumulators packed
          3-subtiles-per-psum-bank, zero-initialized by one start=True
          matmul against a zeroed tile so every real AV matmul is an
          order-independent start=False accumulation)
        local AV: same, with E masked to same-bar keys (host-shipped
          128-aligned mask band per chunk).
      Readout per half: strided reciprocal over the packed denominator
      columns, gate fold, rescale+combine -> comb [q, 64] bf16; batched
      DMA-XBAR transpose -> comb^T; output projection [q 128, 512] per
      subtile; psum->sbuf copy; batched DMA out. Half-0's projection is
      spread across half-1's chunk loop to keep PE/Act dense.
  - Host: sum the 8 partial outputs + bo + Wo@bv.

DMA count is minimized (each dma_start costs ~625ns of serialized HWDGE
dispatch): 12 input DMAs, 6 XBAR transposes, 6 output DMAs.
"""

import numpy as np

S = 2048
D = 512
H = 8
DH = 64
SCALE = 1.0 / np.sqrt(DH)
NCHUNK = S // 128       # 16 key chunks of 128
NHALF = 2               # query halves of 1024
QHALF = S // NHALF
NSUB = QHALF // 128     # 8 query subtiles per half
MB = 640                # aligned mask band width per chunk
ACC_GROUPS = [(0, 1, 2), (3, 4, 5), (6, 7)]
WALL_W = 1280           # wqk 512 | wv 256 | wot 512


def _legalize_waits(nc, mybir):
    """This walrus codegen accepts at most ONE sync wait per instruction.
    Split any instruction carrying N>1 waits into N-1 preceding single-wait
    NoOps on the same engine (waits execute in order on the sequencer)."""
    ctr = 0
    for f in nc.m.functions:
        for b in f.blocks:
            insts = b.instructions
            if not any(i.sync_info and len(i.sync_info.on_wait) > 1 for i in insts):
                continue
            new = []
            for ins in insts:
                si = ins.sync_info
                if si is not None and len(si.on_wait) > 1:
                    waits = list(si.on_wait)
                    for w in waits[:-1]:
                        ctr += 1
                        nop = mybir.InstNoOp(name=f"waitsplit-{ctr}", engine=ins.engine)
                        nop.sync_info = mybir.SyncInfo(on_wait=[w], on_update=[])
                        new.append(nop)
                    ins.sync_info = mybir.SyncInfo(
                        on_wait=[waits[-1]], on_update=list(si.on_update)
                    )
                new.append(ins)
            insts.clear()
            insts.extend(new)
    return ctr


def _bar_bounds(bp):
    """bp: sorted int array [S] -> list of (start, end) per bar."""
    change = np.nonzero(np.diff(bp))[0] + 1
    starts = np.concatenate([[0], change])
    ends = np.concatenate([change, [len(bp)]])
    return list(zip(starts.tolist(), ends.tolist()))


def _abands(bars):
    """Per chunk: 128-aligned query band of bars intersecting the chunk."""
    ab = []
    for c in range(NCHUNK):
        klo, khi = c * 128, (c + 1) * 128
        bs = [b for b in bars if b[1] > klo and b[0] < khi]
        blo, bhi = bs[0][0], bs[-1][1]
        alo = (blo // 128) * 128
        ahi = ((bhi + 127) // 128) * 128
        assert ahi - alo <= MB
        ab.append((alo, ahi))
    return ab


def _build(bars):
    import concourse.bass as bass
    import concourse.tile as tile
    import concourse.mybir as mybir

    dt = mybir.dt
    AF = mybir.ActivationFunctionType
    OP = mybir.AluOpType
    f32 = dt.float32
    f32r = dt.float32r
    bf16 = dt.bfloat16

    nc = bass.Bass()
    xt_d = nc.dram_tensor("xt", [D, S], bf16, kind="ExternalInput")
    wall_d = nc.dram_tensor("wall", [128, WALL_W], bf16, kind="ExternalInput")
    smalls_d = nc.dram_tensor("smalls", [128, 8], f32, kind="ExternalInput")
    mask_d = nc.dram_tensor("maskband", [128, NCHUNK * MB], bf16, kind="ExternalInput")
    out_d = nc.dram_tensor("out_partial", [S, D], f32, kind="ExternalOutput")

    aband = _abands(bars)
    lav = {}  # (hq, c) -> (s_lo, s_hi, [global subtile indices])
    sub_chunks = {}  # global subtile -> chunks writing its L region
    for hq in range(NHALF):
        qlo, qhi = hq * QHALF, (hq + 1) * QHALF
        for c in range(NCHUNK):
            alo, ahi = aband[c]
            s_lo, s_hi = max(alo, qlo), min(ahi, qhi)
            if s_lo >= s_hi:
                continue
            subs = list(range(s_lo // 128, s_hi // 128))
            lav[(hq, c)] = (s_lo, s_hi, subs)
            for sg in subs:
                sub_chunks.setdefault(sg, []).append(c)
    assert all(sg in sub_chunks for sg in range(NHALF * NSUB))

    def acc_region(acc_tiles, s_loc):
        for t, grp in enumerate(ACC_GROUPS):
            if s_loc in grp:
                return acc_tiles[t], grp.index(s_loc) * 130, t
        raise AssertionError

    with tile.TileContext(nc, pool_alloc_mode="queue") as tc:
        with (
            tc.tile_pool(name="persist", bufs=1) as p_keep,
            tc.tile_pool(name="outb", bufs=1) as p_out,
        ):
            qt = p_keep.tile([DH, S], f32r, tag="qt")
            kt = p_keep.tile([DH, S], f32r, tag="kt")
            vt = [p_keep.tile([128, DH + 1], bf16, tag=f"vt{c}", name=f"vt{c}")
                  for c in range(NCHUNK)]
            maskt = p_keep.tile([128, NCHUNK * MB], bf16, tag="maskt")
            smalls = p_keep.tile([128, 8], f32, tag="smalls")
            wall = p_keep.tile([128, WALL_W], bf16, tag="wall")
            zl = p_keep.tile([128, 390], bf16, tag="zl")
            outbuf = p_out.tile([128, NHALF * NSUB * D], f32, tag="outbuf")
            wqks = [wall[:, kc * 128:(kc + 1) * 128] for kc in range(4)]
            wvs = [wall[:, 512 + kc * DH:512 + (kc + 1) * DH] for kc in range(4)]
            wot = wall[0:DH, 768:768 + D]

            # ---------------- input DMAs ----------------
            nc.sync.dma_start(wall[:], wall_d[:])
            with (
                tc.tile_pool(name="inp", bufs=1) as p_in,
                tc.tile_pool(name="ps", bufs=2, space="PSUM") as p_s,
                tc.tile_pool(name="pacc", bufs=1, space="PSUM") as p_acc,
                tc.tile_pool(name="pop", bufs=1, space="PSUM") as p_op,
                tc.tile_pool(name="pe", bufs=6) as p_e,
                tc.tile_pool(name="pel", bufs=2) as p_el,
                tc.tile_pool(name="pcomb", bufs=2) as p_comb,
                tc.tile_pool(name="pct", bufs=2) as p_ct,
                tc.tile_pool(name="prr", bufs=3) as p_rr,
                tc.tile_pool(name="pt1", bufs=2) as p_t1,
            ):
                xts = [p_in.tile([128, S], bf16, tag=f"xt{i}", name=f"xts{i}")
                       for i in range(4)]
                for p in range(2):
                    for kc in range(4):
                        nc.sync.dma_start(
                            xts[kc][:, p * 1024:(p + 1) * 1024],
                            xt_d[kc * 128:(kc + 1) * 128, p * 1024:(p + 1) * 1024],
                        )
                nc.sync.dma_start(
                    maskt[:, 0:4 * MB], mask_d[:, 0:4 * MB]
                )
                nc.sync.dma_start(
                    maskt[:, 4 * MB:NCHUNK * MB], mask_d[:, 4 * MB:NCHUNK * MB]
                )
                nc.sync.dma_start(smalls[:], smalls_d[:])
                # zeros + ones-column init (Pool engine, no deps)
                nc.gpsimd.memset(zl[:], 0.0)
                for c in range(NCHUNK):
                    nc.gpsimd.memset(vt[c][:, DH:DH + 1], 1.0)

                # ---------------- projections (psum shared with attention:
                # QK windows ride the scores pool, V chunks the op pool) ----
                def qk_window(w):
                    ps = p_s.tile([128, QHALF], f32, tag="s", name=f"qkps{w}")
                    for kc in range(4):
                        nc.tensor.matmul(
                            ps[:, 0:512],
                            wqks[kc],
                            xts[kc][:, w * 512:(w + 1) * 512],
                            start=(kc == 0),
                            stop=(kc == 3),
                        )
                    # w0/w1 q-copies ride the still-idle Act engine
                    if w < 2:
                        nc.scalar.copy(qt[:, w * 512:(w + 1) * 512], ps[0:64, 0:512])
                    else:
                        nc.vector.tensor_copy(
                            qt[:, w * 512:(w + 1) * 512], ps[0:64, 0:512]
                        )
                    nc.vector.tensor_copy(
                        kt[:, w * 512:(w + 1) * 512], ps[64:128, 0:512]
                    )

                def v_chunk(c):
                    pv = p_op.tile([128, D], f32, tag="op", name=f"vps{c}")
                    for kc in range(4):
                        nc.tensor.matmul(
                            pv[:, 0:DH],
                            xts[kc][:, c * 128:(c + 1) * 128],
                            wvs[kc],
                            start=(kc == 0),
                            stop=(kc == 3),
                        )
                    nc.vector.tensor_copy(vt[c][:, 0:DH], pv[:, 0:DH])
                halves = []  # per half: (acc_tiles, comb, combT, rrs)

                def emit_init(hq, acc_tiles):
                    for t, g in enumerate(ACC_GROUPS):
                        w = 130 * len(g)
                        nc.tensor.matmul(
                            acc_tiles[t][:, 0:w],
                            zl[:, 0:128],
                            zl[:, 0:w],
                            start=True,
                            stop=False,
                            skip_group_check=True,
                        )

                def emit_scores_exp(hq, c):
                    qlo = hq * QHALF
                    sc = p_s.tile([128, QHALF], f32, tag="s", name=f"sc{hq}_{c}")
                    for n in range(QHALF // 512):
                        nc.tensor.matmul(
                            sc[:, n * 512:(n + 1) * 512],
                            kt[:, c * 128:(c + 1) * 128],
                            qt[:, qlo + n * 512:qlo + (n + 1) * 512],
                            start=True,
                            stop=True,
                        )
                    ec = p_e.tile([128, QHALF], bf16, tag="e", name=f"ec{hq}_{c}")
                    nc.scalar.activation(ec[:], sc[:], AF.Exp)
                    return ec

                def emit_av(hq, c, ec, acc_tiles):
                    qlo = hq * QHALF
                    for sl in range(NSUB):
                        at, off, _ = acc_region(acc_tiles, sl)
                        nc.tensor.matmul(
                            at[:, off:off + 65],
                            ec[:, sl * 128:(sl + 1) * 128],
                            vt[c][:],
                            start=False,
                            stop=(c == NCHUNK - 1),
                            skip_group_check=True,
                        )
                    if (hq, c) in lav:
                        s_lo, s_hi, subs = lav[(hq, c)]
                        alo, _ = aband[c]
                        w = s_hi - s_lo
                        el = p_el.tile([128, MB], bf16, tag="el")
                        nc.vector.tensor_mul(
                            el[:, 0:w],
                            ec[:, s_lo - qlo:s_hi - qlo],
                            maskt[:, c * MB + (s_lo - alo):c * MB + (s_hi - alo)],
                        )
                        for sg in subs:
                            sl = sg - hq * NSUB
                            at, off, _ = acc_region(acc_tiles, sl)
                            nc.tensor.matmul(
                                at[:, off + 65:off + 130],
                                el[:, sg * 128 - s_lo:sg * 128 - s_lo + 128],
                                vt[c][:],
                                start=False,
                                stop=(c == sub_chunks[sg][-1]),
                                skip_group_check=True,
                            )

                def emit_rescale(hq):
                    # hq==1: the G*r multiply rides the now-idle Act engine
                    # so the tail's serial DVE chain halves
                    acc_tiles, comb, _, rrs = halves[hq]
                    for t, g in enumerate(ACC_GROUPS):
                        w = 130 * len(g)
                        rr = p_rr.tile([128, 6], f32, tag="rr", name=f"rr{hq}_{t}")
                        nc.vector.reciprocal(
                            rr[:, 0:2 * len(g)], acc_tiles[t][:, 64:w:65]
                        )
                        nc.vector.tensor_mul(
                            rr[:, 0:2 * len(g)], rr[:, 0:2 * len(g)],
                            smalls[:, 0:2 * len(g)]
                        )
                        rrs.append(rr)
                    for sl in range(NSUB):
                        at, off, t = acc_region(acc_tiles, sl)
                        rr = rrs[t]
                        kk = sl - (0, 3, 6)[t]
                        t1 = p_t1.tile([128, DH], f32, tag="t1")
                        if hq == 1:
                            nc.scalar.activation(
                                t1[:], at[:, off:off + DH], AF.Identity,
                                scale=rr[:, 2 * kk:2 * kk + 1],
                            )
                        else:
                            nc.vector.tensor_scalar_mul(
                                t1[:], at[:, off:off + DH], rr[:, 2 * kk:2 * kk + 1]
                            )
                        nc.vector.scalar_tensor_tensor(
                            comb[:, sl * 128:sl * 128 + DH],
                            at[:, off + 65:off + 65 + DH],
                            rr[:, 2 * kk + 1:2 * kk + 2],
                            t1[:],
                            OP.mult,
                            OP.add,
                        )

                def emit_warm(n, rhs, w, wd):
                    # keep the PE p-state hot across known idle windows:
                    # dependency-free (or gated by `rhs`) throwaway matmuls
                    warm = p_op.tile([128, D], f32, tag="op", name=f"warm{wd}")
                    if rhs is None:
                        rhs = zl[:, 0:w]
                    for i in range(n):
                        nc.tensor.matmul(
                            warm[:, 0:w],
                            zl[:, 0:128],
                            rhs,
                            start=True,
                            stop=True,
                        )

                def emit_xbar(hq, sl0, nsl):
                    _, comb, combT, _ = halves[hq]
                    nc.sync.dma_start_transpose(
                        combT[:, sl0 * 128:(sl0 + nsl) * 128].rearrange(
                            "p (j c) -> p j c", j=nsl
                        ),
                        comb[:, sl0 * 128:(sl0 + nsl) * 128],
                    )

                def emit_proj(hq, sl, op_tile, copy_eng):
                    _, _, combT, _ = halves[hq]
                    sg = hq * NSUB + sl
                    nc.tensor.matmul(
                        op_tile[:, 0:D],
                        combT[0:DH, sl * 128:(sl + 1) * 128],
                        wot,
                        start=True,
                        stop=True,
                    )
                    if copy_eng == "act":
                        nc.scalar.copy(outbuf[:, sg * D:(sg + 1) * D], op_tile[:, 0:D])
                    else:
                        nc.vector.tensor_copy(
                            outbuf[:, sg * D:(sg + 1) * D], op_tile[:, 0:D]
                        )

                def emit_outdma(hq, sl0, nsl):
                    # gpsimd (SWDGE) queue: keeps the serialized HWDGE path
                    # free for input DMAs and XBAR transposes
                    g0 = hq * NSUB + sl0
                    dst = out_d[g0 * 128:(g0 + nsl) * 128, :].rearrange(
                        "(j p) c -> p j c", p=128
                    )
                    src = outbuf[:, g0 * D:(g0 + nsl) * D].rearrange(
                        "p (j c) -> p j c", j=nsl
                    )
                    nc.gpsimd.dma_start(dst, src)

                def new_half(hq):
                    acc_tiles = [
                        p_acc.tile([128, 130 * len(g)], f32, tag=f"acc{t}",
                                   name=f"acc{t}_{hq}")
                        for t, g in enumerate(ACC_GROUPS)
                    ]
                    comb = p_comb.tile([128, NSUB * 128], bf16, tag="comb",
                                       name=f"comb{hq}")
                    combT = p_ct.tile([128, NSUB * 128], bf16, tag="ct",
                                      name=f"combT{hq}")
                    nc.gpsimd.memset(comb[:], 0.0)
                    halves.append((acc_tiles, comb, combT, []))
                    return acc_tiles

                # ---- half 0 (projections interleaved with the chunk loop:
                # V runs 2 chunks ahead through the op pool; QK windows 2/3
                # slot into the scores pool once xt panel 1 has landed) ----
                acc0 = new_half(0)
                emit_init(0, acc0)
                emit_warm(12, None, 390, "start")
                qk_window(0)
                qk_window(1)
                v_chunk(0)
                v_chunk(1)
                for c in range(NCHUNK):
                    ec = emit_scores_exp(0, c)
                    emit_av(0, c, ec, acc0)
                    if c + 2 < NCHUNK:
                        v_chunk(c + 2)
                    if c == 0:
                        qk_window(2)
                    if c == 1:
                        qk_window(3)
                emit_rescale(0)

                # ---- half 1, with half-0 projection spread through it ----
                acc1 = new_half(1)
                pend = []  # deferred AV blocks before init (init waits on
                # half-0 accumulator release; keep PE/Act fed meanwhile)
                for c in range(5):
                    pend.append((c, emit_scores_exp(1, c)))
                # PE filler during the half-0 rescale drain (gated on the
                # first comb0 write so it runs in the gap, not earlier)
                emit_warm(6, halves[0][1][:, 0:128], 128, "mid")
                emit_init(1, acc1)
                for c, ec in pend:
                    emit_av(1, c, ec, acc1)
                for c in range(5, NCHUNK):
                    ec = emit_scores_exp(1, c)
                    emit_av(1, c, ec, acc1)
                    # interleave half-0 projection: one subtile per chunk.
                    # All XBARs precede the first out-DMA: the framework
                    # serializes later same-phase DMAs behind SWDGE
                    # completions, so an out-DMA before an XBAR would stall
                    # the XBAR (and the PE projection behind it) for ~6us.
                    if c == 5:
                        emit_xbar(0, 0, 4)
                    if c == 9:
                        emit_xbar(0, 4, 4)
                    if 6 <= c <= 13:
                        sl = c - 6
                        op = p_op.tile([128, D], f32, tag="op", name=f"op0_{sl}")
                        emit_proj(0, sl, op, "dve")
                        if sl % 4 == 3:
                            emit_outdma(0, sl - 3, 4)

                # ---- half-1 readout (tail) ----
                emit_rescale(1)
                # PE filler while the rescale drains
                emit_warm(8, halves[1][1][:, 0:128], 128, "tail")
                for sl in range(0, NSUB, 2):
                    emit_xbar(1, sl, 2)
                for sl in range(NSUB):
                    # rotate 3 psum buffers: op pool + two score-pool tiles
                    if sl % 3 == 0:
                        op = p_op.tile([128, D], f32, tag="op", name=f"op1_{sl}")
                    else:
                        op = p_s.tile([128, QHALF], f32, tag="s", name=f"ops1_{sl}")
                    emit_proj(1, sl, op, "act" if sl % 2 == 0 else "dve")
                    if sl % 2 == 1:
                        emit_outdma(1, sl - 1, 2)

    _legalize_waits(nc, mybir)
    return nc


_CACHE = {}


def _get_built(bar_key, bars):
    if bar_key not in _CACHE:
        _CACHE[bar_key] = _build(bars)
    return _CACHE[bar_key]


def _np_reference(hidden_states, bar_positions, attention_mask, Wq, bq, Wk, bk,
                  Wv, bv, Wo, bo, bar_emb, gate):
    """Plain numpy fallback (only used if inputs violate baked assumptions)."""
    B, S_, _ = hidden_states.shape
    x = hidden_states.astype(np.float64)
    q = (x @ Wq.T + bq).reshape(B, S_, H, DH).transpose(0, 2, 1, 3)
    k = (x @ Wk.T + bk).reshape(B, S_, H, DH).transpose(0, 2, 1, 3)
    v = (x @ Wv.T + bv).reshape(B, S_, H, DH).transpose(0, 2, 1, 3)
    scores = np.einsum("bhqd,bhkd->bhqk", q, k) * SCALE
    pad = attention_mask[:, None, None, :]
    bar_mask = (bar_positions[:, :, None] == bar_positions[:, None, :])[:, None]
    NEG = -np.inf

    def softmax(s):
        s = s - s.max(-1, keepdims=True)
        e = np.exp(s)
        return e / e.sum(-1, keepdims=True)

    local = softmax(np.where(bar_mask & pad, scores, NEG))
    emb = bar_emb[np.asarray(bar_positions) % bar_emb.shape[0]]
    bias = np.sum(emb * emb, axis=-1)
    glob = softmax(np.where(pad, scores + bias[:, None, :, None], NEG))
    la = np.einsum("bhqk,bhkd->bhqd", local, v)
    ga = np.einsum("bhqk,bhkd->bhqd", glob, v)
    g = 1.0 / (1.0 + np.exp(-gate))[None, :, None, None]
    comb = g * la + (1.0 - g) * ga
    out = comb.transpose(0, 2, 1, 3).reshape(B, S_, H * DH)
    return (out @ Wo.T + bo).astype(np.float32)


def kernel(**inputs):
    import ml_dtypes

    bf = ml_dtypes.bfloat16
    hidden_states = np.asarray(inputs["hidden_states"], dtype=np.float32)
    bar_positions = np.asarray(inputs["bar_positions"])
    attention_mask = np.asarray(inputs["attention_mask"])
    Wq = np.asarray(inputs["Wq"], dtype=np.float32)
    bq = np.asarray(inputs["bq"], dtype=np.float32)
    Wk = np.asarray(inputs["Wk"], dtype=np.float32)
    bk = np.asarray(inputs["bk"], dtype=np.float32)
    Wv = np.asarray(inputs["Wv"], dtype=np.float32)
    bv = np.asarray(inputs["bv"], dtype=np.float32)
    Wo = np.asarray(inputs["Wo"], dtype=np.float32)
    bo = np.asarray(inputs["bo"], dtype=np.float32)
    gate = np.asarray(inputs["gate"], dtype=np.float32)

    bp = bar_positions[0].astype(np.int64)
    if (
        hidden_states.shape != (1, S, D)
        or not bool(attention_mask.all())
        or not bool((np.diff(bp) >= 0).all())
        or np.any(bq)
        or np.any(bk)
    ):
        return _np_reference(
            hidden_states, bar_positions, attention_mask, Wq, bq, Wk, bk,
            Wv, bv, Wo, bo, np.asarray(inputs["bar_emb"], dtype=np.float32), gate,
        )

    bars = _bar_bounds(bp)
    nc = _get_built(bp.tobytes(), bars)

    # aligned mask bands (same for every core)
    aband = _abands(bars)
    maskband = np.zeros((128, NCHUNK * MB), dtype=bf)
    for c in range(NCHUNK):
        alo, ahi = aband[c]
        eq = (bp[c * 128:(c + 1) * 128, None] == bp[None, alo:ahi])
        maskband[:, c * MB:c * MB + (ahi - alo)] = eq.astype(bf)

    xt = np.ascontiguousarray(hidden_states[0].T).astype(bf)  # [512, 2048]
    g = 1.0 / (1.0 + np.exp(-gate.astype(np.float64)))  # sigmoid, [H]
    in_maps = []
    for h in range(H):
        sl = slice(h * DH, (h + 1) * DH)
        wall = np.zeros((128, WALL_W), dtype=np.float32)
        for kc in range(4):
            r = slice(kc * 128, (kc + 1) * 128)
            wall[:, kc * 128:kc * 128 + 64] = Wq[sl, r].T * np.float32(SCALE)
            wall[:, kc * 128 + 64:(kc + 1) * 128] = Wk[sl, r].T
            wall[:, 512 + kc * DH:512 + (kc + 1) * DH] = Wv[sl, r].T
        wall[0:DH, 768:768 + D] = Wo[:, sl].T
        smalls = np.zeros((128, 8), dtype=np.float32)
        smalls[:, 0:6:2] = np.float32(1.0 - g[h])
        smalls[:, 1:6:2] = np.float32(g[h])
        in_maps.append({
            "xt": xt,
            "wall": wall.astype(bf),
            "smalls": smalls,
            "maskband": maskband,
        })

    res = _run_spmd(nc, in_maps)
    out = np.zeros((S, D), dtype=np.float32)
    for h in range(H):
        out += res.results[h]["out_partial"]
    out += bo + Wo @ bv
    return out.reshape(1, S, D)


def _run_spmd(nc, in_maps, **kw):
    from concourse.bass_utils import run_bass_kernel_spmd

    return run_bass_kernel_spmd(nc, in_maps, list(range(H)), **kw)


# revision 23
# speedup vs baseline: 1.0955x; 1.0955x over previous
"""Bar-level attention Trainium2 kernel (8 NeuronCores, head-parallel).

Contract: kernel(**inputs) takes the FULL inputs from setup_inputs() and
returns the FULL [1, 2048, 512] float32 output.

Strategy (one head per core, 8 heads / 8 cores), v2 layout:
  - Host: XT [512, 2048] bf16; one packed weight wall per head
    ([WqT*scale | WkT] | WvT | WoT) bf16; sigmoid(gate) folded into
    per-partition scalars; bq/bk are zero in setup_inputs (guarded);
    bv/bo folded into the host-side bias add (exact: softmax rows sum
    to 1, so the V bias passes straight through both branches).
  - Device (per core):
      Q^T/K^T [64, 2048] f32r (joint [128, *] projection, split on copy),
      V [k, 65] bf16 per 128-key chunk (col 64 = ones -> denominators).
      Attention per 1024-query half, per 128-key chunk:
        S^T = K_chunk @ Q^T -> psum [128, 1024] (keys on partitions)
        E = exp(S^T) -> bf16 (no max subtraction: scores ~ N(0,1))
        global AV: acc[q_sub 128, 65] += E_sub^T @ V (8 query subtiles,
          full 128-partition contraction; G/L accumulators packed
          3-subtiles-per-psum-bank, zero-initialized by one start=True
          matmul against a zeroed tile so every real AV matmul is an
          order-independent start=False accumulation)
        local AV: same, with E masked to same-bar keys (host-shipped
          128-aligned mask band per chunk).
      Readout per half: strided reciprocal over the packed denominator
      columns, gate fold, rescale+combine -> comb [q, 64] bf16; batched
      DMA-XBAR transpose -> comb^T; output projection [q 128, 512] per
      subtile; psum->sbuf copy; batched DMA out. Half-0's projection is
      spread across half-1's chunk loop to keep PE/Act dense.
  - Host: sum the 8 partial outputs + bo + Wo@bv.

DMA count is minimized (each dma_start costs ~625ns of serialized HWDGE
dispatch): 12 input DMAs, 6 XBAR transposes, 6 output DMAs.
"""

import numpy as np

S = 2048
D = 512
H = 8
DH = 64
SCALE = 1.0 / np.sqrt(DH)
NCHUNK = S // 128       # 16 key chunks of 128
NHALF = 2               # query halves of 1024
QHALF = S // NHALF
NSUB = QHALF // 128     # 8 query subtiles per half
MB = 640                # aligned mask band width per chunk
ACC_GROUPS = [(0, 1, 2), (3, 4, 5), (6, 7)]
WALL_W = 1280           # wqk 512 | wv 256 | wot 512


def _legalize_waits(nc, mybir):
    """This walrus codegen accepts at most ONE sync wait per instruction.
    Split any instruction carrying N>1 waits into N-1 preceding single-wait
    NoOps on the same engine (waits execute in order on the sequencer)."""
    ctr = 0
    for f in nc.m.functions:
        for b in f.blocks:
            insts = b.instructions
            if not any(i.sync_info and len(i.sync_info.on_wait) > 1 for i in insts):
                continue
            new = []
            for ins in insts:
                si = ins.sync_info
                if si is not None and len(si.on_wait) > 1:
                    waits = list(si.on_wait)
                    for w in waits[:-1]:
                        ctr += 1
                        nop = mybir.InstNoOp(name=f"waitsplit-{ctr}", engine=ins.engine)
                        nop.sync_info = mybir.SyncInfo(on_wait=[w], on_update=[])
                        new.append(nop)
                    ins.sync_info = mybir.SyncInfo(
                        on_wait=[waits[-1]], on_update=list(si.on_update)
                    )
                new.append(ins)
            insts.clear()
            insts.extend(new)
    return ctr


def _bar_bounds(bp):
    """bp: sorted int array [S] -> list of (start, end) per bar."""
    change = np.nonzero(np.diff(bp))[0] + 1
    starts = np.concatenate([[0], change])
    ends = np.concatenate([change, [len(bp)]])
    return list(zip(starts.tolist(), ends.tolist()))


def _abands(bars):
    """Per chunk: 128-aligned query band of bars intersecting the chunk."""
    ab = []
    for c in range(NCHUNK):
        klo, khi = c * 128, (c + 1) * 128
        bs = [b for b in bars if b[1] > klo and b[0] < khi]
        blo, bhi = bs[0][0], bs[-1][1]
        alo = (blo // 128) * 128
        ahi = ((bhi + 127) // 128) * 128
        assert ahi - alo <= MB
        ab.append((alo, ahi))
    return ab


def _build(bars):
    import concourse.bass as bass
    import concourse.tile as tile
    import concourse.mybir as mybir

    dt = mybir.dt
    AF = mybir.ActivationFunctionType
    OP = mybir.AluOpType
    f32 = dt.float32
    f32r = dt.float32r
    bf16 = dt.bfloat16

    nc = bass.Bass()
    xt_d = nc.dram_tensor("xt", [D, S], bf16, kind="ExternalInput")
    wall_d = nc.dram_tensor("wall", [128, WALL_W], bf16, kind="ExternalInput")
    smalls_d = nc.dram_tensor("smalls", [128, 8], f32, kind="ExternalInput")
    mask_d = nc.dram_tensor("maskband", [128, NCHUNK * MB], bf16, kind="ExternalInput")
    out_d = nc.dram_tensor("out_partial", [S, D], bf16, kind="ExternalOutput")

    aband = _abands(bars)
    lav = {}  # (hq, c) -> (s_lo, s_hi, [global subtile indices])
    sub_chunks = {}  # global subtile -> chunks writing its L region
    for hq in range(NHALF):
        qlo, qhi = hq * QHALF, (hq + 1) * QHALF
        for c in range(NCHUNK):
            alo, ahi = aband[c]
            s_lo, s_hi = max(alo, qlo), min(ahi, qhi)
            if s_lo >= s_hi:
                continue
            subs = list(range(s_lo // 128, s_hi // 128))
            lav[(hq, c)] = (s_lo, s_hi, subs)
            for sg in subs:
                sub_chunks.setdefault(sg, []).append(c)
    assert all(sg in sub_chunks for sg in range(NHALF * NSUB))

    def acc_region(acc_tiles, s_loc):
        for t, grp in enumerate(ACC_GROUPS):
            if s_loc in grp:
                return acc_tiles[t], grp.index(s_loc) * 130, t
        raise AssertionError

    with tile.TileContext(nc, pool_alloc_mode="queue") as tc:
        with (
            tc.tile_pool(name="persist", bufs=1) as p_keep,
            tc.tile_pool(name="outb", bufs=1) as p_out,
        ):
            qt = p_keep.tile([DH, S], f32r, tag="qt")
            kt = p_keep.tile([DH, S], f32r, tag="kt")
            vt = [p_keep.tile([128, DH + 1], bf16, tag=f"vt{c}", name=f"vt{c}")
                  for c in range(NCHUNK)]
            maskt = p_keep.tile([128, NCHUNK * MB], bf16, tag="maskt")
            smalls = p_keep.tile([128, 8], f32, tag="smalls")
            wall = p_keep.tile([128, WALL_W], bf16, tag="wall")
            zl = p_keep.tile([128, 390], bf16, tag="zl")
            outbuf = p_out.tile([128, NHALF * NSUB * D], bf16, tag="outbuf")
            wqks = [wall[:, kc * 128:(kc + 1) * 128] for kc in range(4)]
            wvs = [wall[:, 512 + kc * DH:512 + (kc + 1) * DH] for kc in range(4)]
            wot = wall[0:DH, 768:768 + D]

            # ---------------- input DMAs ----------------
            nc.sync.dma_start(wall[:], wall_d[:])
            with (
                tc.tile_pool(name="inp", bufs=1) as p_in,
                tc.tile_pool(name="ps", bufs=2, space="PSUM") as p_s,
                tc.tile_pool(name="pacc", bufs=1, space="PSUM") as p_acc,
                tc.tile_pool(name="pop", bufs=1, space="PSUM") as p_op,
                tc.tile_pool(name="pe", bufs=6) as p_e,
                tc.tile_pool(name="pel", bufs=2) as p_el,
                tc.tile_pool(name="pcomb", bufs=2) as p_comb,
                tc.tile_pool(name="pct", bufs=2) as p_ct,
                tc.tile_pool(name="prr", bufs=3) as p_rr,
                tc.tile_pool(name="pt1", bufs=2) as p_t1,
            ):
                xts = [p_in.tile([128, S], bf16, tag=f"xt{i}", name=f"xts{i}")
                       for i in range(4)]

                def xt_panel(p):
                    for kc in range(4):
                        nc.sync.dma_start(
                            xts[kc][:, p * 512:(p + 1) * 512],
                            xt_d[kc * 128:(kc + 1) * 128, p * 512:(p + 1) * 512],
                        )
                xt_panel(0)
                xt_panel(1)
                nc.sync.dma_start(
                    maskt[:, 0:4 * MB], mask_d[:, 0:4 * MB]
                )
                xt_panel(2)
                xt_panel(3)
                nc.sync.dma_start(
                    maskt[:, 4 * MB:NCHUNK * MB], mask_d[:, 4 * MB:NCHUNK * MB]
                )
                nc.sync.dma_start(smalls[:], smalls_d[:])
                # zeros + ones-column init (Pool engine, no deps)
                nc.gpsimd.memset(zl[:], 0.0)
                for c in range(NCHUNK):
                    nc.gpsimd.memset(vt[c][:, DH:DH + 1], 1.0)

                # ---------------- projections (psum shared with attention:
                # QK windows ride the scores pool, V chunks the op pool) ----
                def qk_window(w):
                    # windows 0/1 prefill through the (idle) scores pool;
                    # 2/3 ride the op/V psum chain so they never steal a
                    # scores buffer mid-pipeline
                    if w < 2:
                        ps = p_s.tile([128, QHALF], f32, tag="s", name=f"qkps{w}")
                    else:
                        ps = p_op.tile([128, D], f32, tag="op", name=f"qkps{w}")
                    for kc in range(4):
                        nc.tensor.matmul(
                            ps[:, 0:512],
                            wqks[kc],
                            xts[kc][:, w * 512:(w + 1) * 512],
                            start=(kc == 0),
                            stop=(kc == 3),
                        )
                    # w0/w1 q-copies ride the still-idle Act engine
                    if w < 2:
                        nc.scalar.copy(qt[:, w * 512:(w + 1) * 512], ps[0:64, 0:512])
                    else:
                        nc.vector.tensor_copy(
                            qt[:, w * 512:(w + 1) * 512], ps[0:64, 0:512]
                        )
                    nc.vector.tensor_copy(
                        kt[:, w * 512:(w + 1) * 512], ps[64:128, 0:512]
                    )

                def v_chunk(c):
                    pv = p_op.tile([128, D], f32, tag="op", name=f"vps{c}")
                    for kc in range(4):
                        nc.tensor.matmul(
                            pv[:, 0:DH],
                            xts[kc][:, c * 128:(c + 1) * 128],
                            wvs[kc],
                            start=(kc == 0),
                            stop=(kc == 3),
                        )
                    nc.vector.tensor_copy(vt[c][:, 0:DH], pv[:, 0:DH])
                halves = []  # per half: (acc_tiles, comb, combT, rrs)

                def emit_init(hq, acc_tiles):
                    for t, g in enumerate(ACC_GROUPS):
                        w = 130 * len(g)
                        nc.tensor.matmul(
                            acc_tiles[t][:, 0:w],
                            zl[:, 0:128],
                            zl[:, 0:w],
                            start=True,
                            stop=False,
                            skip_group_check=True,
                        )

                def emit_scores_exp(hq, c):
                    qlo = hq * QHALF
                    sc = p_s.tile([128, QHALF], f32, tag="s", name=f"sc{hq}_{c}")
                    for n in range(QHALF // 512):
                        nc.tensor.matmul(
                            sc[:, n * 512:(n + 1) * 512],
                            kt[:, c * 128:(c + 1) * 128],
                            qt[:, qlo + n * 512:qlo + (n + 1) * 512],
                            start=True,
                            stop=True,
                        )
                    ec = p_e.tile([128, QHALF], bf16, tag="e", name=f"ec{hq}_{c}")
                    nc.scalar.activation(ec[:], sc[:], AF.Exp)
                    return ec

                def emit_av(hq, c, ec, acc_tiles):
                    qlo = hq * QHALF
                    for sl in range(NSUB):
                        at, off, _ = acc_region(acc_tiles, sl)
                        nc.tensor.matmul(
                            at[:, off:off + 65],
                            ec[:, sl * 128:(sl + 1) * 128],
                            vt[c][:],
                            start=False,
                            stop=(c == NCHUNK - 1),
                            skip_group_check=True,
                        )
                    if (hq, c) in lav:
                        s_lo, s_hi, subs = lav[(hq, c)]
                        alo, _ = aband[c]
                        w = s_hi - s_lo
                        el = p_el.tile([128, MB], bf16, tag="el")
                        nc.vector.tensor_mul(
                            el[:, 0:w],
                            ec[:, s_lo - qlo:s_hi - qlo],
                            maskt[:, c * MB + (s_lo - alo):c * MB + (s_hi - alo)],
                        )
                        for sg in subs:
                            sl = sg - hq * NSUB
                            at, off, _ = acc_region(acc_tiles, sl)
                            nc.tensor.matmul(
                                at[:, off + 65:off + 130],
                                el[:, sg * 128 - s_lo:sg * 128 - s_lo + 128],
                                vt[c][:],
                                start=False,
                                stop=(c == sub_chunks[sg][-1]),
                                skip_group_check=True,
                            )

                def emit_rescale(hq):
                    # hq==1: the G*r multiply rides the now-idle Act engine
                    # so the tail's serial DVE chain halves
                    acc_tiles, comb, _, rrs = halves[hq]
                    for t, g in enumerate(ACC_GROUPS):
                        w = 130 * len(g)
                        rr = p_rr.tile([128, 6], f32, tag="rr", name=f"rr{hq}_{t}")
                        nc.vector.reciprocal(
                            rr[:, 0:2 * len(g)], acc_tiles[t][:, 64:w:65]
                        )
                        nc.vector.tensor_mul(
                            rr[:, 0:2 * len(g)], rr[:, 0:2 * len(g)],
                            smalls[:, 0:2 * len(g)]
                        )
                        rrs.append(rr)
                    for sl in range(NSUB):
                        at, off, t = acc_region(acc_tiles, sl)
                        rr = rrs[t]
                        kk = sl - (0, 3, 6)[t]
                        t1 = p_t1.tile([128, DH], f32, tag="t1")
                        if hq == 1:
                            nc.scalar.activation(
                                t1[:], at[:, off:off + DH], AF.Identity,
                                scale=rr[:, 2 * kk:2 * kk + 1],
                            )
                        else:
                            nc.vector.tensor_scalar_mul(
                                t1[:], at[:, off:off + DH], rr[:, 2 * kk:2 * kk + 1]
                            )
                        nc.vector.scalar_tensor_tensor(
                            comb[:, sl * 128:sl * 128 + DH],
                            at[:, off + 65:off + 65 + DH],
                            rr[:, 2 * kk + 1:2 * kk + 2],
                            t1[:],
                            OP.mult,
                            OP.add,
                        )

                def emit_warm(n, rhs, w, wd):
                    # keep the PE p-state hot across known idle windows:
                    # dependency-free (or gated by `rhs`) throwaway matmuls
                    warm = p_op.tile([128, D], f32, tag="op", name=f"warm{wd}")
                    if rhs is None:
                        rhs = zl[:, 0:w]
                    for i in range(n):
                        nc.tensor.matmul(
                            warm[:, 0:w],
                            zl[:, 0:128],
                            rhs,
                            start=True,
                            stop=True,
                        )

                def emit_xbar(hq, sl0, nsl):
                    _, comb, combT, _ = halves[hq]
                    nc.sync.dma_start_transpose(
                        combT[:, sl0 * 128:(sl0 + nsl) * 128].rearrange(
                            "p (j c) -> p j c", j=nsl
                        ),
                        comb[:, sl0 * 128:(sl0 + nsl) * 128],
                    )

                def emit_proj(hq, sl, op_tile, copy_eng):
                    _, _, combT, _ = halves[hq]
                    sg = hq * NSUB + sl
                    nc.tensor.matmul(
                        op_tile[:, 0:D],
                        combT[0:DH, sl * 128:(sl + 1) * 128],
                        wot,
                        start=True,
                        stop=True,
                    )
                    if copy_eng == "act":
                        nc.scalar.copy(outbuf[:, sg * D:(sg + 1) * D], op_tile[:, 0:D])
                    else:
                        nc.vector.tensor_copy(
                            outbuf[:, sg * D:(sg + 1) * D], op_tile[:, 0:D]
                        )

                def emit_outdma(hq, sl0, nsl):
                    # sync queue, always emitted after this half's XBARs:
                    # the scheduler hands DMA-queue tokens to later DMAs,
                    # so an out-DMA scheduled before an XBAR stalls it
                    g0 = hq * NSUB + sl0
                    dst = out_d[g0 * 128:(g0 + nsl) * 128, :].rearrange(
                        "(j p) c -> p j c", p=128
                    )
                    src = outbuf[:, g0 * D:(g0 + nsl) * D].rearrange(
                        "p (j c) -> p j c", j=nsl
                    )
                    nc.sync.dma_start(dst, src)

                def new_half(hq):
                    acc_tiles = [
                        p_acc.tile([128, 130 * len(g)], f32, tag=f"acc{t}",
                                   name=f"acc{t}_{hq}")
                        for t, g in enumerate(ACC_GROUPS)
                    ]
                    comb = p_comb.tile([128, NSUB * 128], bf16, tag="comb",
                                       name=f"comb{hq}")
                    combT = p_ct.tile([128, NSUB * 128], bf16, tag="ct",
                                      name=f"combT{hq}")
                    nc.gpsimd.memset(comb[:], 0.0)
                    halves.append((acc_tiles, comb, combT, []))
                    return acc_tiles

                # ---- half 0 (projections interleaved with the chunk loop:
                # V runs 2 chunks ahead through the op pool; QK windows 2/3
                # slot into the scores pool once xt panel 1 has landed) ----
                acc0 = new_half(0)
                emit_init(0, acc0)
                emit_warm(12, None, 390, "start")
                qk_window(0)
                qk_window(1)
                v_chunk(0)
                v_chunk(1)
                for c in range(NCHUNK):
                    ec = emit_scores_exp(0, c)
                    emit_av(0, c, ec, acc0)
                    if c + 2 < NCHUNK:
                        v_chunk(c + 2)
                    if c == 0:
                        qk_window(2)
                    if c == 1:
                        qk_window(3)
                emit_rescale(0)

                # ---- half 1, with half-0 projection spread through it ----
                acc1 = new_half(1)
                pend = []  # deferred AV blocks before init (init waits on
                # half-0 accumulator release; keep PE/Act fed meanwhile)
                for c in range(5):
                    pend.append((c, emit_scores_exp(1, c)))
                # PE filler during the half-0 rescale drain (gated on the
                # first comb0 write so it runs in the gap, not earlier)
                emit_warm(6, halves[0][1][:, 0:128], 128, "mid")
                emit_init(1, acc1)
                for c, ec in pend:
                    emit_av(1, c, ec, acc1)
                for c in range(5, NCHUNK):
                    ec = emit_scores_exp(1, c)
                    emit_av(1, c, ec, acc1)
                    # interleave half-0 projection: one subtile per chunk.
                    # All XBARs precede the first out-DMA: the framework
                    # serializes later same-phase DMAs behind SWDGE
                    # completions, so an out-DMA before an XBAR would stall
                    # the XBAR (and the PE projection behind it) for ~6us.
                    if c == 5:
                        emit_xbar(0, 0, 4)
                    if c == 9:
                        emit_xbar(0, 4, 4)
                    if 6 <= c <= 13:
                        sl = c - 6
                        op = p_op.tile([128, D], f32, tag="op", name=f"op0_{sl}")
                        emit_proj(0, sl, op, "dve")
                        if sl % 4 == 3:
                            emit_outdma(0, sl - 3, 4)

                # ---- half-1 readout (tail) ----
                emit_rescale(1)
                # PE filler while the rescale drains
                emit_warm(8, halves[1][1][:, 0:128], 128, "tail")
                for sl in range(0, NSUB, 2):
                    emit_xbar(1, sl, 2)
                for sl in range(NSUB):
                    # rotate 3 psum buffers: op pool + two score-pool tiles
                    if sl % 3 == 0:
                        op = p_op.tile([128, D], f32, tag="op", name=f"op1_{sl}")
                    else:
                        op = p_s.tile([128, QHALF], f32, tag="s", name=f"ops1_{sl}")
                    emit_proj(1, sl, op, "act" if sl % 2 == 0 else "dve")
                    if sl % 2 == 1:
                        emit_outdma(1, sl - 1, 2)

    _legalize_waits(nc, mybir)
    return nc


_CACHE = {}


def _get_built(bar_key, bars):
    if bar_key not in _CACHE:
        _CACHE[bar_key] = _build(bars)
    return _CACHE[bar_key]


def _np_reference(hidden_states, bar_positions, attention_mask, Wq, bq, Wk, bk,
                  Wv, bv, Wo, bo, bar_emb, gate):
    """Plain numpy fallback (only used if inputs violate baked assumptions)."""
    B, S_, _ = hidden_states.shape
    x = hidden_states.astype(np.float64)
    q = (x @ Wq.T + bq).reshape(B, S_, H, DH).transpose(0, 2, 1, 3)
    k = (x @ Wk.T + bk).reshape(B, S_, H, DH).transpose(0, 2, 1, 3)
    v = (x @ Wv.T + bv).reshape(B, S_, H, DH).transpose(0, 2, 1, 3)
    scores = np.einsum("bhqd,bhkd->bhqk", q, k) * SCALE
    pad = attention_mask[:, None, None, :]
    bar_mask = (bar_positions[:, :, None] == bar_positions[:, None, :])[:, None]
    NEG = -np.inf

    def softmax(s):
        s = s - s.max(-1, keepdims=True)
        e = np.exp(s)
        return e / e.sum(-1, keepdims=True)

    local = softmax(np.where(bar_mask & pad, scores, NEG))
    emb = bar_emb[np.asarray(bar_positions) % bar_emb.shape[0]]
    bias = np.sum(emb * emb, axis=-1)
    glob = softmax(np.where(pad, scores + bias[:, None, :, None], NEG))
    la = np.einsum("bhqk,bhkd->bhqd", local, v)
    ga = np.einsum("bhqk,bhkd->bhqd", glob, v)
    g = 1.0 / (1.0 + np.exp(-gate))[None, :, None, None]
    comb = g * la + (1.0 - g) * ga
    out = comb.transpose(0, 2, 1, 3).reshape(B, S_, H * DH)
    return (out @ Wo.T + bo).astype(np.float32)


def kernel(**inputs):
    import ml_dtypes

    bf = ml_dtypes.bfloat16
    hidden_states = np.asarray(inputs["hidden_states"], dtype=np.float32)
    bar_positions = np.asarray(inputs["bar_positions"])
    attention_mask = np.asarray(inputs["attention_mask"])
    Wq = np.asarray(inputs["Wq"], dtype=np.float32)
    bq = np.asarray(inputs["bq"], dtype=np.float32)
    Wk = np.asarray(inputs["Wk"], dtype=np.float32)
    bk = np.asarray(inputs["bk"], dtype=np.float32)
    Wv = np.asarray(inputs["Wv"], dtype=np.float32)
    bv = np.asarray(inputs["bv"], dtype=np.float32)
    Wo = np.asarray(inputs["Wo"], dtype=np.float32)
    bo = np.asarray(inputs["bo"], dtype=np.float32)
    gate = np.asarray(inputs["gate"], dtype=np.float32)

    bp = bar_positions[0].astype(np.int64)
    if (
        hidden_states.shape != (1, S, D)
        or not bool(attention_mask.all())
        or not bool((np.diff(bp) >= 0).all())
        or np.any(bq)
        or np.any(bk)
    ):
        return _np_reference(
            hidden_states, bar_positions, attention_mask, Wq, bq, Wk, bk,
            Wv, bv, Wo, bo, np.asarray(inputs["bar_emb"], dtype=np.float32), gate,
        )

    bars = _bar_bounds(bp)
    nc = _get_built(bp.tobytes(), bars)

    # aligned mask bands (same for every core)
    aband = _abands(bars)
    maskband = np.zeros((128, NCHUNK * MB), dtype=bf)
    for c in range(NCHUNK):
        alo, ahi = aband[c]
        eq = (bp[c * 128:(c + 1) * 128, None] == bp[None, alo:ahi])
        maskband[:, c * MB:c * MB + (ahi - alo)] = eq.astype(bf)

    xt = np.ascontiguousarray(hidden_states[0].T).astype(bf)  # [512, 2048]
    g = 1.0 / (1.0 + np.exp(-gate.astype(np.float64)))  # sigmoid, [H]
    in_maps = []
    for h in range(H):
        sl = slice(h * DH, (h + 1) * DH)
        wall = np.zeros((128, WALL_W), dtype=np.float32)
        for kc in range(4):
            r = slice(kc * 128, (kc + 1) * 128)
            wall[:, kc * 128:kc * 128 + 64] = Wq[sl, r].T * np.float32(SCALE)
            wall[:, kc * 128 + 64:(kc + 1) * 128] = Wk[sl, r].T
            wall[:, 512 + kc * DH:512 + (kc + 1) * DH] = Wv[sl, r].T
        wall[0:DH, 768:768 + D] = Wo[:, sl].T
        smalls = np.zeros((128, 8), dtype=np.float32)
        smalls[:, 0:6:2] = np.float32(1.0 - g[h])
        smalls[:, 1:6:2] = np.float32(g[h])
        in_maps.append({
            "xt": xt,
            "wall": wall.astype(bf),
            "smalls": smalls,
            "maskband": maskband,
        })

    res = _run_spmd(nc, in_maps)
    out = np.zeros((S, D), dtype=np.float32)
    for h in range(H):
        out += np.asarray(res.results[h]["out_partial"], dtype=np.float32)
    out += bo + Wo @ bv
    return out.reshape(1, S, D)


def _run_spmd(nc, in_maps, **kw):
    from concourse.bass_utils import run_bass_kernel_spmd

    return run_bass_kernel_spmd(nc, in_maps, list(range(H)), **kw)
